# revision 4
# baseline (speedup 1.0000x reference)
"""Trainium2 Bass kernel for nn_AttentionModel (B=8, S=2048, D=1024).

Strategy: data-parallel over batch — core b computes batch b entirely
locally (no collectives).

Math restructure (softmax is invariant to per-row constants):
  scores[q,k] = Q[q]·K[k] = x1_q (Wq^T Wk) x2_k^T + alpha_q + beta_k + c
  alpha_q + c drop out in softmax;  M2 = Wq^T Wk is precomputed on host,
  beta_k = x2_k · (Wk^T bq) folds into the per-key exp bias column.
  So the K projection vanishes and the Q projection becomes H1 = x1 @ M2.

Per-core dataflow (fp8e4 DoubleRow matmuls, fp32 PSUM accumulate):
  warmup: ~16 junk matmuls warm the PE HAM clock gate during the DMA head
  phase A: H1^T tiles = M2s.T @ x1^T   (DR fp8, M2s = 32*M2 in fp8)
           V[s,d] = x3t.T @ wvt (+bv)  (bf16; the +V residual dominates
           the output so it needs bf16 accuracy) -> vres bf16 + v2 fp8
           (DR-paired k rows, with a ones column for the denominator)
  phase B per 512-wide q-chunk:
    scoresT[k,q] tiles = x2p.T @ h1t   (DR fp8)
    es = exp(SCALE/32 * psum + bias_k) (bias = SCALE*beta + key mask)
    per 128-query tile: po = es.T @ v2 (DR), pd = es.T @ ones
    out[q,:] = po[q,:]/pd[q] + vres[q,:]  -> DMA out (bf16)

DoubleRow layout: operands are 3D APs [128, 2, n] — partition p, pair
half i covers contraction index 256*j + 128*i + p for chunk j.  All
device inputs are host-packed into [128, W] monoliths so each tensor
loads with one or two big DMAs (per-DMA issue on an engine queue costs
~650ns; the boot-critical m2/x1 head DMAs issue from scalar/vector
queues which come out of the kernel preamble earlier than sync).
"""

import numpy as np

B, S, D = 8, 2048, 1024
P = 128
NQ = 512                 # moving free dim per matmul output
N_QCHUNK = S // NQ       # 4
KT_TILES = S // P        # 16 key tiles of 128
KT2 = KT_TILES // 2      # 8 DR key chunks of 256
DJ = D // 256            # 4 DR contraction chunks over d
VW = 1040                # v2 per-half width: 1024 d cols + ones col + pad
SCALE = 1.0 / float(np.sqrt(D))
M2_SCALE = 32.0
NEG_MASK = -30000.0


def _apply_tile_patch():
    """This walrus build allows at most ONE semaphore wait on the tail
    CTRL/Drain instruction; Tile's kernel-tail drain carries one wait per
    touched logical proc. Spread them over multiple drains."""
    import copy

    from concourse import tile as _tile
    from concourse.vector_clock import ScopedClock as _ScopedClock

    if getattr(_tile.TileContext, "_drain_patch_applied", False):
        return

    def _patched(self, tick_clock, wait_clock):
        nc = self.nc
        drain_inst = nc.sync.drain()
        wait_clock.add_sem_waits(
            drain_inst.ins, _ScopedClock({None: tick_clock.global_clock})
        )
        mi = drain_inst.ins
        si = mi.sync_info
        waits = list(si.on_wait) if (si is not None and si.on_wait) else []
        if len(waits) > 1:
            si.on_wait = waits[:1]
            mi.sync_info = si
            for i in range(1, len(waits)):
                extra = nc.sync.drain()
                esi = copy.copy(si)
                esi.on_wait = [waits[i]]
                esi.on_update = []
                extra.ins.sync_info = esi

        nc.all_engine_barrier()
        assert self.sems is not None
        popped = nc._tile_sem_poison_stack.pop()
        assert popped is self._sem_poison
        nc.clear_and_free_semaphores(list(self.sems.allocated().values()))
        nc.all_engine_barrier()

    _tile.TileContext._drain_and_barrier = _patched
    _tile.TileContext._drain_patch_applied = True


def _split_excess_waits(nc, max_waits=1):
    """This walrus build rejects instructions carrying more than one
    semaphore wait ("Too many sync wait commands"). Hoist extra waits onto
    same-engine NoOp carriers inserted right before the instruction."""
    from concourse import mybir

    n_split = 0
    for f in nc.m.functions:
        for blk in f.blocks:
            insts = list(blk.instructions)
            out = []
            changed = False
            for inst in insts:
                si = inst.sync_info
                waits = list(si.on_wait) if (si is not None and si.on_wait) else []
                if len(waits) > max_waits:
                    head, tail = waits[:-max_waits], waits[-max_waits:]
                    for i in range(0, len(head), max_waits):
                        carrier = mybir.InstNoOp(
                            name=nc.get_next_instruction_name(),
                            engine=inst.engine,
                            ins=[],
                            outs=[],
                            sync_info=mybir.SyncInfo(
                                on_wait=head[i : i + max_waits], on_update=[]
                            ),
                        )
                        out.append(carrier)
                    si.on_wait = tail
                    inst.sync_info = si
                    changed = True
                    n_split += 1
                out.append(inst)
            if changed:
                blk.instructions = out
    return n_split


def _install_neff_cache():
    """walrus compile of this kernel is slow; cache the NEFF keyed on the
    BIR json hash so repeat runs (same graph) skip it."""
    import hashlib
    import os
    import shutil

    from concourse import bass2jax, bass_utils

    if getattr(bass_utils, "_neff_cache_installed", False):
        return
    orig = bass_utils.compile_bir_kernel

    def cached(bir_json, tmpdir, neff_name="file.neff"):
        h = hashlib.sha256(bytes(bir_json)).hexdigest()[:32]
        cdir = os.path.expanduser("~/.bass-neff-cache")
        os.makedirs(cdir, exist_ok=True)
        cpath = os.path.join(cdir, h + ".neff")
        if os.path.exists(cpath):
            dst = os.path.join(tmpdir, neff_name)
            shutil.copyfile(cpath, dst)
            return dst
        p = orig(bir_json, tmpdir, neff_name)
        try:
            shutil.copyfile(p, cpath)
        except OSError:
            pass
        return p

    bass_utils.compile_bir_kernel = cached
    bass2jax.compile_bir_kernel = cached
    bass_utils._neff_cache_installed = True


def _ap3(t_ap, pstride, col_off, half_step, n):
    """3D DoubleRow AP [128, 2, n] over an SBUF tile: partition stride
    pstride, halves half_step elements apart, n contiguous elements."""
    import concourse.bass as bass

    return bass.AP(
        tensor=t_ap.tensor,
        offset=t_ap.offset + col_off,
        ap=[[pstride, P], [half_step, 2], [1, n]],
    )


def build_nc(split_waits=True):
    """Build the per-core Bass graph (SPMD: same graph on all 8 cores)."""
    import concourse.bass as bass
    import concourse.tile as tile
    from concourse import mybir

    _apply_tile_patch()

    f32 = mybir.dt.float32
    bf16 = mybir.dt.bfloat16
    f8 = mybir.dt.float8e4
    AF = mybir.ActivationFunctionType
    DR = mybir.MatmulPerfMode.DoubleRow

    nc = bass.Bass()

    # Host-packed monolith inputs: [128 partitions, W] each.
    x1p = nc.dram_tensor("x1p", [P, DJ * 2 * S], f8, kind="ExternalInput")
    x2p = nc.dram_tensor("x2p", [P, DJ * 2 * S], f8, kind="ExternalInput")
    m2p = nc.dram_tensor("m2p", [P, DJ * 2 * D], f8, kind="ExternalInput")
    x3p = nc.dram_tensor("x3p", [P, 8 * S], bf16, kind="ExternalInput")
    wvp = nc.dram_tensor("wvp", [P, 8 * D], bf16, kind="ExternalInput")
    bvr = nc.dram_tensor("bvr", [D], f32, kind="ExternalInput")
    biasp = nc.dram_tensor("biasp", [P, KT_TILES], f32, kind="ExternalInput")
    out = nc.dram_tensor("out", [S, D], bf16, kind="ExternalOutput")

    with tile.TileContext(nc) as tc:
        with (
            tc.tile_pool(name="persist", bufs=1) as persist,
            tc.tile_pool(name="consts", bufs=1) as consts,
        ):
            # Persistent SBUF monoliths.
            m2sb = persist.tile([P, DJ * 2 * D], f8, tag="m2sb", name="m2sb")
            x1sb = persist.tile([P, DJ * 2 * S], f8, tag="x1sb", name="x1sb")
            x2sb = persist.tile([P, DJ * 2 * S], f8, tag="x2sb", name="x2sb")
            h1sb = persist.tile([P, DJ * 2 * S], f8, tag="h1sb", name="h1sb")
            wvsb = persist.tile([P, 8 * D], bf16, tag="wvsb", name="wvsb")
            x3sb = persist.tile([P, 8 * S], bf16, tag="x3sb", name="x3sb")
            v2 = [
                persist.tile([P, 2 * VW], f8, tag=f"v2_{i}", name=f"v2_{i}")
                for i in range(KT2)
            ]
            vres = [
                persist.tile([P, D], bf16, tag=f"vr{i}", name=f"vr{i}")
                for i in range(KT_TILES)
            ]

            bias_sb = consts.tile([P, KT_TILES], f32, tag="bias")
            bv_sb = consts.tile([P, D], f32, tag="bv")
            warm = consts.tile([P, NQ], bf16, tag="warm")

            # Boot-critical DMAs on scalar/vector queues (they clear the
            # kernel preamble earlier than sync); everything else on sync
            # in consumption order.
            nc.scalar.dma_start(out=m2sb[:, 0 : 2 * D], in_=m2p[:, 0 : 2 * D])
            nc.gpsimd.dma_start(out=x1sb[:, 0 : 2 * S], in_=x1p[:, 0 : 2 * S])
            nc.sync.dma_start(
                out=m2sb[:, 2 * D :], in_=m2p[:, 2 * D :]
            )
            nc.sync.dma_start(out=x1sb[:, 2 * S :], in_=x1p[:, 2 * S :])
            nc.sync.dma_start(out=wvsb[:], in_=wvp[:, :])
            nc.sync.dma_start(out=x3sb[:], in_=x3p[:, :])
            nc.sync.dma_start(out=x2sb[:], in_=x2p[:, :])
            nc.sync.dma_start(out=bias_sb[:], in_=biasp[:, :])
            bvr_ap = bvr[:]
            bv_bcast = bass.AP(
                tensor=bvr_ap.tensor, offset=bvr_ap.offset, ap=[[0, P], [1, D]]
            )
            nc.sync.dma_start(out=bv_sb[:], in_=bv_bcast)

            # ones columns of v2 (softmax denominator source)
            for i in range(KT2):
                for h in range(2):
                    nc.vector.memset(v2[i][:, h * VW + D : h * VW + D + 1], 1.0)

            # HAM warmup: keep the PE busy while the head DMAs land so the
            # clock gate is at 8/8 when real matmuls start.
            nc.vector.memset(warm[:], 0.0)
            with tc.tile_pool(name="psW", bufs=1, space="PSUM") as psW:
                pw = psW.tile([P, NQ], f32, tag="pw", name="pw")
                for _ in range(16):
                    nc.tensor.matmul(
                        pw[:], lhsT=warm[:, 0:P], rhs=warm[:], start=True, stop=True
                    )

            # ---------------- Phase A: H1 projection + V ----------------
            with tc.tile_pool(name="psA", bufs=4, space="PSUM") as psA:
                # --- H1^T[do, s] tiles = sum_dj M2s[dj].T @ x1T[dj] ---
                for sc in range(N_QCHUNK):
                    for do in range(D // P):
                        ps = psA.tile([P, NQ], f32, tag="psA", name="psA_t")
                        for dj in range(DJ):
                            nc.tensor.matmul(
                                ps[:],
                                lhsT=_ap3(
                                    m2sb[:], DJ * 2 * D, dj * 2 * D + do * P, D, P
                                ),
                                rhs=_ap3(
                                    x1sb[:], DJ * 2 * S, dj * 2 * S + sc * NQ, S, NQ
                                ),
                                start=(dj == 0),
                                stop=(dj == DJ - 1),
                                perf_mode=DR,
                            )
                        nc.scalar.activation(
                            out=h1sb[
                                :,
                                (do // 2) * 2 * S + (do % 2) * S + sc * NQ :
                                (do // 2) * 2 * S + (do % 2) * S + (sc + 1) * NQ,
                            ],
                            in_=ps[:],
                            func=AF.Identity,
                            scale=1.0,
                        )

                # --- V[s, d] tiles = x3t.T @ wvt (+bv), bf16 ---
                for si in range(KT_TILES):
                    ps2 = [
                        psA.tile([P, NQ], f32, tag="psA", name="psA_t")
                        for _ in range(2)
                    ]
                    for ii in range(8):
                        lhsT = x3sb[:, ii * S + si * P : ii * S + (si + 1) * P]
                        for dc in range(2):
                            nc.tensor.matmul(
                                ps2[dc][:],
                                lhsT=lhsT,
                                rhs=wvsb[:, ii * D + dc * NQ : ii * D + (dc + 1) * NQ],
                                start=(ii == 0),
                                stop=(ii == 7),
                            )
                    for dc in range(2):
                        sl = slice(dc * NQ, (dc + 1) * NQ)
                        nc.vector.tensor_add(
                            out=vres[si][:, sl], in0=ps2[dc][:], in1=bv_sb[:, sl]
                        )
                        nc.scalar.activation(
                            out=v2[si // 2][
                                :, (si % 2) * VW + dc * NQ : (si % 2) * VW + (dc + 1) * NQ
                            ],
                            in_=vres[si][:, sl],
                            func=AF.Identity,
                            scale=1.0,
                        )

            # ---------------- Phase B: attention ----------------
            with (
                tc.tile_pool(name="es", bufs=KT2 + 2) as es_pool,
                tc.tile_pool(name="outp", bufs=4) as out_pool,
                tc.tile_pool(name="recp", bufs=4) as rec_pool,
                tc.tile_pool(name="psS", bufs=2, space="PSUM") as psS,
                tc.tile_pool(name="psO", bufs=4, space="PSUM") as psO,
                tc.tile_pool(name="psD", bufs=2, space="PSUM") as psD,
            ):
                for qc in range(N_QCHUNK):
                    # scoresT tiles [k 128, q 512] -> exp -> fp8 es pairs
                    es_tiles = []
                    for kt2 in range(KT2):
                        es = es_pool.tile([P, 2 * NQ], f8, tag="es", name="es_t")
                        for h in range(2):
                            kt = 2 * kt2 + h
                            ps = psS.tile([P, NQ], f32, tag="psS", name="psS_t")
                            for dj in range(DJ):
                                nc.tensor.matmul(
                                    ps[:],
                                    lhsT=_ap3(
                                        x2sb[:], DJ * 2 * S, dj * 2 * S + kt * P, S, P
                                    ),
                                    rhs=_ap3(
                                        h1sb[:], DJ * 2 * S, dj * 2 * S + qc * NQ, S, NQ
                                    ),
                                    start=(dj == 0),
                                    stop=(dj == DJ - 1),
                                    perf_mode=DR,
                                )
                            nc.scalar.activation(
                                out=es[:, h * NQ : (h + 1) * NQ],
                                in_=ps[:],
                                func=AF.Exp,
                                bias=bias_sb[:, kt : kt + 1],
                                scale=SCALE / M2_SCALE,
                            )
                        es_tiles.append(es)

                    # attn @ V for the 4 query tiles of this chunk
                    for qi in range(NQ // P):
                        qg = qc * (NQ // P) + qi
                        po = [
                            psO.tile([P, NQ], f32, tag="po", name=f"psO_{dc}")
                            for dc in range(2)
                        ]
                        pd = psD.tile([P, 1], f32, tag="pd", name="pd_t")
                        for kt2 in range(KT2):
                            lhsT = _ap3(es_tiles[kt2][:], 2 * NQ, qi * P, NQ, P)
                            st = (kt2 == 0)
                            sp = (kt2 == KT2 - 1)
                            for dc in range(2):
                                nc.tensor.matmul(
                                    po[dc][:],
                                    lhsT=lhsT,
                                    rhs=_ap3(v2[kt2][:], 2 * VW, dc * NQ, VW, NQ),
                                    start=st,
                                    stop=sp,
                                    perf_mode=DR,
                                )
                            nc.tensor.matmul(
                                pd[:],
                                lhsT=lhsT,
                                rhs=_ap3(v2[kt2][:], 2 * VW, D, VW, 1),
                                start=st,
                                stop=sp,
                                perf_mode=DR,
                            )
                        rec = rec_pool.tile([P, 1], f32, tag="rec", name="rec_t")
                        nc.vector.reciprocal(out=rec[:], in_=pd[:])
                        for dc in range(2):
                            sl = slice(dc * NQ, (dc + 1) * NQ)
                            ob = out_pool.tile([P, NQ], bf16, tag="ob", name="ob_t")
                            nc.scalar.activation(
                                out=ob[:], in_=po[dc][:], func=AF.Copy,
                                bias=0.0, scale=rec[:],
                            )
                            nc.vector.tensor_add(
                                out=ob[:], in0=ob[:], in1=vres[qg][:, sl]
                            )
                            nc.sync.dma_start(
                                out=out[qg * P : (qg + 1) * P, sl], in_=ob[:]
                            )

    if split_waits:
        _split_excess_waits(nc)
    return nc


def _pack_dr(a):
    """[D, C] -> [128, (D//256)*2*C]: contraction row 256j+128i+p lands at
    partition p, col block (j, i) — DoubleRow pairing, one flat tile."""
    Din, C = a.shape
    return np.ascontiguousarray(
        a.reshape(Din // 256, 2, P, C).transpose(2, 0, 1, 3).reshape(P, -1)
    )


def _pack_rows(a):
    """[D, C] -> [128, (D//128)*C]: row 128i+p lands at partition p, col
    block i — plain row grouping, one flat tile."""
    Din, C = a.shape
    return np.ascontiguousarray(
        a.reshape(Din // P, P, C).transpose(1, 0, 2).reshape(P, -1)
    )


def _prep_inputs(plms1, plms2, plms3, seqlengths, Wq, bq, Wk, bk, Wv, bv):
    """Host-side shard + layout prep. Returns in_maps for 8 cores."""
    import ml_dtypes

    bf = ml_dtypes.bfloat16
    f8 = ml_dtypes.float8_e4m3
    f32 = np.float32

    def to_f8(a):
        return np.clip(a, -240.0, 240.0).astype(f8)

    Wq, Wk, Wv = np.asarray(Wq, f32), np.asarray(Wk, f32), np.asarray(Wv, f32)
    bq, bk, bv = np.asarray(bq, f32), np.asarray(bk, f32), np.asarray(bv, f32)

    M2 = (Wq.T @ Wk).astype(f32)            # scores = x1 @ M2 @ x2^T + beta
    m2p = _pack_dr(to_f8(M2_SCALE * M2))
    betav = Wk.T @ bq                        # beta_k = x2_k . betav
    wvp = _pack_rows(np.ascontiguousarray(Wv.T).astype(bf))
    bvr = bv
    seqlengths = np.asarray(seqlengths)

    in_maps = []
    ar = np.arange(S)
    for b in range(B):
        x1b = np.asarray(plms1[b], f32)
        x2b = np.asarray(plms2[b], f32)
        x3b = np.asarray(plms3[b], f32)
        beta = x2b @ betav                   # [S]
        bias = SCALE * beta + np.where(ar < int(seqlengths[b]), 0.0, NEG_MASK)
        biasp = np.ascontiguousarray(bias.astype(f32).reshape(KT_TILES, P).T)
        in_maps.append(
            {
                "x1p": _pack_dr(to_f8(x1b.T)),
                "x2p": _pack_dr(to_f8(x2b.T)),
                "m2p": m2p,
                "x3p": _pack_rows(np.ascontiguousarray(x3b.T).astype(bf)),
                "wvp": wvp,
                "bvr": bvr,
                "biasp": biasp,
            }
        )
    return in_maps


def kernel(**inputs) -> np.ndarray:
    from concourse.bass_utils import run_bass_kernel_spmd

    _install_neff_cache()

    in_maps = _prep_inputs(
        inputs["plms1"], inputs["plms2"], inputs["plms3"], inputs["seqlengths"],
        inputs["Wq"], inputs["bq"], inputs["Wk"], inputs["bk"],
        inputs["Wv"], inputs["bv"],
    )
    nc = build_nc()
    res = run_bass_kernel_spmd(nc, in_maps, core_ids=list(range(B)))
    return np.stack(
        [np.asarray(res.results[i]["out"], np.float32) for i in range(B)]
    )
